# revision 11
# baseline (speedup 1.0000x reference)
"""Trainium2 Bass kernel for nn_CausalAttention (B=1, P=S=2048, D=1024, H=16).

Strategy (8 NeuronCores, SPMD, no collectives):
  - Queries sharded round-robin: core c owns rows p = c + 8k, k in [0,256).
    This balances the causal softmax/ctx work exactly across cores and makes
    the device program identical on every core (all data-dependence lives in
    per-core host-prepared arrays).
  - Each core: replicated k/v projections (full K,V), q projection for its
    rows, scores = q@k^T*scale + prev (prev pre-loaded into PSUM via an
    identity matmul, qk accumulated on top), channel gate applied as a
    per-partition tensor_scalar during PSUM->SBUF copy-out, mask applied as
    an additive -1e9 bias over the staircase region, unshifted exp with
    fused row-sum (accum_out), attn = e * (1/den), PE-transposed attn blocks
    feed the attn@v matmuls, then the output projection.
  - float32 storage for outputs; matmul operand tiles are float32r (full PE
    rate; the BIR verifier requires operands to be produced as f32r).
  - Outputs are compact per-core; host scatters into full arrays. Regions a
    core never writes (masked attn tail) rely on run_bass_kernel_spmd's
    pre-zeroed output buffers.
"""

import sys

sys.path.insert(0, "/opt/trn_rl_repo")

import numpy as np

import concourse.bass as bass
import concourse.mybir as mybir
import concourse.tile as tile
from concourse.bass_utils import run_bass_kernel_spmd
from concourse.masks import make_identity
from concourse.vector_clock import ScopedClock, VectorClock

B, P, S, D, H = 1, 2048, 2048, 1024, 16
DH = D // H
D_IN, SEQ_LEN = 8, 256
NCORES = 8
KPC = P // NCORES          # rows per core (256)
SCALE = DH ** -0.5
F32 = mybir.dt.float32
F32R = mybir.dt.float32r
ACT_COPY = mybir.ActivationFunctionType.Copy
ACT_EXP = mybir.ActivationFunctionType.Exp
NEG = -1e9


class _ChunkedDrainTileContext(tile.TileContext):
    """Tail drain emitted as one-wait-per-drain chunks.

    The walrus build here rejects CTRL-class instructions carrying more than
    one sync-wait ("Too many sync wait commands"); Tile's stock exit attaches
    every outstanding proc's wait to a single drain.
    """

    def _drain_and_barrier(self, tick_clock, wait_clock):
        gc = tick_clock.global_clock
        n = len(gc)
        procs = [p for p in range(n) if gc[p] > 0]
        for _ in range(2):
            for p in procs:
                vec = [gc[q] if q == p else 0 for q in range(n)]
                d = self.nc.sync.drain()
                wait_clock.add_sem_waits(
                    d.ins, ScopedClock({None: VectorClock(vec)})
                )
        self.nc.sync.drain()
        self.nc.all_engine_barrier()
        assert self.sems is not None
        popped = self.nc._tile_sem_poison_stack.pop()
        assert popped is self._sem_poison
        self.nc.clear_and_free_semaphores(list(self.sems.allocated().values()))
        self.nc.all_engine_barrier()


def _ld(ap):
    """DRAM-side f32r view for DMA into an f32r tile (bit-identical)."""
    return ap.bitcast(F32R)


def _split_multi_waits(bir: dict) -> dict:
    """Split instructions carrying >1 sync-wait into single-wait NOPs.

    This walrus build rejects any instruction with more than one sync-wait
    command. A sequence of same-engine wait-only NOPs immediately before the
    instruction is semantically identical (the engine executes its stream in
    order, so all waits are satisfied before the instruction runs).
    """
    ctr = 0
    for f in bir["functions"]:
        for bb in f["blocks"]:
            insts = bb["instructions"]
            if not any(
                len((i.get("sync_info") or {}).get("on_wait") or ()) > 1
                for i in insts
            ):
                continue
            out = []
            for inst in insts:
                si = inst.get("sync_info")
                waits = si.get("on_wait") if si else None
                if waits and len(waits) > 1 and inst["engine"] != "Unassigned":
                    for w in waits[:-1]:
                        ctr += 1
                        out.append(
                            {
                                "debug": inst.get("debug", 0),
                                "engine": inst["engine"],
                                "ins": [],
                                "name": f"I-waitfix-{ctr}",
                                "opcode": "NoOp",
                                "outs": [],
                                "sync_info": {"on_wait": [w], "on_update": []},
                            }
                        )
                    si["on_wait"] = [waits[-1]]
                out.append(inst)
            bb["instructions"] = out
    return bir


def _install_waitfix(nc):
    import orjson

    orig = nc.to_json_bytes

    def fixed():
        return orjson.dumps(_split_multi_waits(orjson.loads(orig())))

    nc.to_json_bytes = fixed
    return nc


def build_program(causal: bool):
    """One SPMD program; every per-core difference is data, not structure."""
    nc = bass.Bass()
    dp = nc.declare_dram_parameter
    qt_d = dp("qt", [D, KPC], F32, isOutput=False)
    kt_d = dp("kt", [D, S], F32, isOutput=False)
    vt_d = dp("vt", [D, S], F32, isOutput=False)
    wqt_d = dp("wqt", [D, D], F32, isOutput=False)
    wkt_d = dp("wkt", [D, D], F32, isOutput=False)
    wvt_d = dp("wvt", [D, D], F32, isOutput=False)
    wot_d = dp("wot", [D, D], F32, isOutput=False)
    prev_d = dp("prev", [2, H, 128, S], F32, isOutput=False)
    gcol_d = dp("gatecol", [2, 128, D_IN], F32, isOutput=False)
    stair_d = dp("stairb", [2, 128, 1024], F32, isOutput=False)
    raw_d = dp("raw", [2, H, 128, S], F32, isOutput=True)
    attn_d = dp("attn", [2, H, 128, S], F32, isOutput=True)
    out_d = dp("out", [2, 128, D], F32, isOutput=True)

    # Allowed width per partition-tile (cols >= W are fully masked for every
    # row of the tile); stair region [1024m, 1024m+1024) carries the additive
    # mask bias.
    W = (1024, 2048) if causal else (2048, 2048)
    NB = (W[0] // 128, W[1] // 128)  # attn blocks feeding ctx, per tile

    with _ChunkedDrainTileContext(nc) as tc:
        with (
            tc.tile_pool(name="persist", bufs=1) as persist,
            tc.tile_pool(name="consts", bufs=1) as consts,
        ):
            ident = consts.tile([128, 128], F32)
            make_identity(nc, ident)
            identR = consts.tile([128, 128], F32R)
            nc.vector.tensor_copy(out=identR, in_=ident)
            zeroR = consts.tile([128, 128], F32R)
            nc.scalar.activation(out=zeroR, in_=ident, func=ACT_COPY, scale=0.0)
            gcol = consts.tile([128, 2, D_IN], F32)
            nc.sync.dma_start(out=gcol, in_=gcol_d.rearrange("m r b -> r m b"))
            stair = consts.tile([128, 2, 1024], F32)
            nc.sync.dma_start(out=stair, in_=stair_d.rearrange("m r j -> r m j"))

            qT = persist.tile([128, 8, KPC], F32R)
            kT = persist.tile([128, 8, S], F32R)
            v = persist.tile([128, S // 128, D], F32R)

            # ---------------- Phase A: projections ----------------
            with (
                tc.tile_pool(name="wpool", bufs=1) as wpool,
                tc.tile_pool(name="apool", bufs=2) as apool,
                tc.tile_pool(name="apsum", bufs=4, space="PSUM") as apsum,
            ):
                # q projection: qT[d, k] = scale * (Wq @ Q_rows^T)
                wsb = wpool.tile([128, 8, D], F32R, tag="w", name="wq")
                nc.sync.dma_start(
                    out=wsb,
                    in_=_ld(wqt_d.rearrange("(ko ki) n -> ki ko n", ki=128)),
                )
                qtsb = apool.tile([128, 8, KPC], F32R, tag="io", name="qtsb")
                nc.sync.dma_start(
                    out=qtsb,
                    in_=_ld(qt_d.rearrange("(ko ki) n -> ki ko n", ki=128)),
                )
                for mo in range(8):
                    ps = apsum.tile([128, 512], F32, tag="ps", name="ps_q")
                    for ko in range(8):
                        nc.tensor.matmul(
                            ps[:, :KPC],
                            lhsT=wsb[:, ko, 128 * mo : 128 * mo + 128],
                            rhs=qtsb[:, ko, :],
                            start=(ko == 0),
                            stop=(ko == 7),
                        )
                    nc.scalar.activation(
                        out=qT[:, mo, :], in_=ps[:, :KPC], func=ACT_COPY,
                        scale=SCALE,
                    )

                # k projection: kT[d, s] = Wk @ K^T
                wsb = wpool.tile([128, 8, D], F32R, tag="w", name="wk_")
                nc.sync.dma_start(
                    out=wsb,
                    in_=_ld(wkt_d.rearrange("(ko ki) n -> ki ko n", ki=128)),
                )
                for so in range(8):
                    ksb = apool.tile([128, 8, 256], F32R, tag="io", name="ksb")
                    nc.sync.dma_start(
                        out=ksb,
                        in_=_ld(
                            kt_d[:, 256 * so : 256 * so + 256].rearrange(
                                "(ko ki) n -> ki ko n", ki=128
                            )
                        ),
                    )
                    for mo in range(8):
                        ps = apsum.tile([128, 512], F32, tag="ps", name="ps_k")
                        for ko in range(8):
                            nc.tensor.matmul(
                                ps[:, :256],
                                lhsT=wsb[:, ko, 128 * mo : 128 * mo + 128],
                                rhs=ksb[:, ko, :],
                                start=(ko == 0),
                                stop=(ko == 7),
                            )
                        nc.any.tensor_copy(
                            out=kT[:, mo, 256 * so : 256 * so + 256],
                            in_=ps[:, :256],
                        )

                # v projection (natural layout): v[s, d] = V @ Wv^T
                wsb = wpool.tile([128, 8, D], F32R, tag="w", name="wv_")
                nc.sync.dma_start(
                    out=wsb,
                    in_=_ld(wvt_d.rearrange("(ko ki) n -> ki ko n", ki=128)),
                )
                for so in range(S // 128):
                    vsb = apool.tile([128, 8, 128], F32R, tag="io", name="vsb")
                    nc.sync.dma_start(
                        out=vsb,
                        in_=_ld(
                            vt_d[:, 128 * so : 128 * so + 128].rearrange(
                                "(ko ki) n -> ki ko n", ki=128
                            )
                        ),
                    )
                    for no in range(2):
                        ps = apsum.tile([128, 512], F32, tag="ps", name="ps_v")
                        for ko in range(8):
                            nc.tensor.matmul(
                                ps,
                                lhsT=vsb[:, ko, :],
                                rhs=wsb[:, ko, 512 * no : 512 * no + 512],
                                start=(ko == 0),
                                stop=(ko == 7),
                            )
                        nc.any.tensor_copy(
                            out=v[:, so, 512 * no : 512 * no + 512], in_=ps
                        )

            # ---------------- Phases B + C ----------------
            with tc.tile_pool(name="bc", bufs=1) as bc:
                ctxT = bc.tile([64, H, KPC], F32R)
                with (
                    tc.tile_pool(name="prevp", bufs=2) as prevp,
                    tc.tile_pool(name="gp", bufs=4) as gp,
                    tc.tile_pool(name="smalls", bufs=4) as smalls,
                    tc.tile_pool(name="atp", bufs=3) as atp,
                    tc.tile_pool(name="spsum", bufs=2, space="PSUM") as spsum,
                    tc.tile_pool(name="tpsum", bufs=2, space="PSUM") as tpsum,
                    tc.tile_pool(name="cpsum", bufs=2, space="PSUM") as cpsum,
                ):
                    for hp in range(8):
                        for hh in range(2):
                            h = 2 * hp + hh
                            po = 64 * hh
                            ctx_ps = cpsum.tile([64, KPC], F32, tag="ctx")
                            e_m = []
                            for m in range(2):
                                g = gp.tile([128, S], F32, tag="g")
                                for half in range(2):
                                    prev_t = prevp.tile(
                                        [128, 1024], F32R, tag="prev",
                                        name="prev_t",
                                    )
                                    nc.sync.dma_start(
                                        out=prev_t,
                                        in_=_ld(
                                            prev_d[m, h][
                                                :,
                                                1024 * half : 1024 * half + 1024,
                                            ]
                                        ),
                                    )
                                    ps = spsum.tile([128, 1024], F32, tag="sc")
                                    for q2 in range(2):
                                        c0 = 1024 * half + 512 * q2
                                        nc.tensor.matmul(
                                            ps[:, 512 * q2 : 512 * q2 + 512],
                                            lhsT=identR,
                                            rhs=prev_t[
                                                :, 512 * q2 : 512 * q2 + 512
                                            ],
                                            start=True,
                                            stop=False,
                                        )
                                        nc.tensor.matmul(
                                            ps[:, 512 * q2 : 512 * q2 + 512],
                                            lhsT=qT[
                                                po : po + 64,
                                                hp,
                                                128 * m : 128 * m + 128,
                                            ],
                                            rhs=kT[
                                                po : po + 64, hp, c0 : c0 + 512
                                            ],
                                            start=False,
                                            stop=True,
                                        )
                                    # gated copy-out, per 256-col gate block
                                    for b2 in range(4):
                                        bg = 4 * half + b2
                                        dst = g[:, 256 * bg : 256 * bg + 256]
                                        src = ps[:, 256 * b2 : 256 * b2 + 256]
                                        sc = gcol[:, m, bg : bg + 1]
                                        if bg % 2 == 0:
                                            nc.vector.tensor_scalar_mul(
                                                dst, src, sc
                                            )
                                        else:
                                            nc.scalar.activation(
                                                out=dst, in_=src,
                                                func=ACT_COPY, scale=sc,
                                            )
                                nc.sync.dma_start(out=raw_d[m, h], in_=g)
                                # additive mask bias over the stair region
                                so_ = 1024 * m if causal else 0
                                nc.vector.tensor_add(
                                    g[:, so_ : so_ + 1024],
                                    g[:, so_ : so_ + 1024],
                                    stair[:, m, :],
                                )
                                den = smalls.tile([128, 1], F32, tag="den")
                                nc.scalar.activation(
                                    out=g[:, : W[m]], in_=g[:, : W[m]],
                                    func=ACT_EXP, accum_out=den,
                                )
                                rec = smalls.tile([128, 1], F32, tag="rec")
                                nc.vector.reciprocal(out=rec, in_=den)
                                nc.any.tensor_scalar_mul(
                                    g[:, : W[m]], g[:, : W[m]], rec
                                )
                                nc.sync.dma_start(
                                    out=attn_d[m, h, :, : W[m]],
                                    in_=g[:, : W[m]],
                                )
                                e_m.append(g)
                            # transpose attn blocks and accumulate attn @ v
                            for j in range(NB[1]):
                                pt = tpsum.tile([128, 256], F32, tag="pt")
                                at = atp.tile([128, 256], F32R, tag="at")
                                if j < NB[0]:
                                    nc.tensor.transpose(
                                        pt[:, 0:128],
                                        e_m[0][:, 128 * j : 128 * j + 128],
                                        ident,
                                    )
                                    nc.tensor.transpose(
                                        pt[:, 128:256],
                                        e_m[1][:, 128 * j : 128 * j + 128],
                                        ident,
                                    )
                                    nc.any.tensor_copy(out=at, in_=pt)
                                else:
                                    nc.gpsimd.tensor_copy(
                                        out=at[:, 0:128], in_=zeroR
                                    )
                                    nc.tensor.transpose(
                                        pt[:, 128:256],
                                        e_m[1][:, 128 * j : 128 * j + 128],
                                        ident,
                                    )
                                    nc.any.tensor_copy(
                                        out=at[:, 128:256], in_=pt[:, 128:256]
                                    )
                                nc.tensor.matmul(
                                    ctx_ps,
                                    lhsT=v[:, j, 64 * h : 64 * h + 64],
                                    rhs=at,
                                    start=(j == 0),
                                    stop=(j == NB[1] - 1),
                                )
                            nc.any.tensor_copy(out=ctxT[:, h, :], in_=ctx_ps)

                # -------------- Phase C: output projection --------------
                with (
                    tc.tile_pool(name="cw", bufs=2) as cw,
                    tc.tile_pool(name="co", bufs=2) as co,
                    tc.tile_pool(name="opsum", bufs=4, space="PSUM") as opsum,
                ):
                    pso = [
                        [
                            opsum.tile([128, 512], F32, tag="po", name="po")
                            for _ in range(2)
                        ]
                        for _ in range(2)
                    ]
                    for h in range(H):
                        wk = cw.tile([64, D], F32R, tag="wo")
                        nc.sync.dma_start(
                            out=wk, in_=_ld(wot_d[64 * h : 64 * h + 64, :])
                        )
                        for m in range(2):
                            for no in range(2):
                                nc.tensor.matmul(
                                    pso[m][no],
                                    lhsT=ctxT[:, h, 128 * m : 128 * m + 128],
                                    rhs=wk[:, 512 * no : 512 * no + 512],
                                    start=(h == 0),
                                    stop=(h == H - 1),
                                )
                    for m in range(2):
                        ot = co.tile([128, D], F32, tag="ot")
                        for no in range(2):
                            nc.any.tensor_copy(
                                out=ot[:, 512 * no : 512 * no + 512],
                                in_=pso[m][no],
                            )
                        nc.sync.dma_start(out=out_d[m], in_=ot)

    return nc


_prog_cache = {}


def _get_program(causal: bool):
    if causal not in _prog_cache:
        _prog_cache[causal] = _install_waitfix(build_program(causal))
    return _prog_cache[causal]


def kernel(Q, K, V, prev, Wq, bq, Wk, bk, Wv, bv, Wo, bo, ch_gate, attn_mask):
    Q, K, V, prev = (np.asarray(t, np.float32) for t in (Q, K, V, prev))
    Wq, Wk, Wv, Wo = (np.asarray(t, np.float32) for t in (Wq, Wk, Wv, Wo))
    ch_gate = np.asarray(ch_gate, np.float32)
    mask2d = np.asarray(attn_mask, bool).reshape(P, S)
    for b in (bq, bk, bv, bo):
        assert not np.any(np.asarray(b)), "nonzero biases not implemented"

    tril = np.tril(np.ones((P, S), bool))
    if np.array_equal(mask2d, tril):
        causal = True
    elif mask2d.all():
        causal = False
    else:
        raise NotImplementedError("attn_mask is neither causal nor all-true")

    gate = (ch_gate >= 0.0).astype(np.float32)  # sigmoid(x) >= 0.5 <=> x >= 0
    kt = np.ascontiguousarray(K[0].T)
    vt = np.ascontiguousarray(V[0].T)
    wqt = np.ascontiguousarray(Wq.T)
    wkt = np.ascontiguousarray(Wk.T)
    wvt = np.ascontiguousarray(Wv.T)
    wot = np.ascontiguousarray(Wo.T)

    in_maps = []
    perms = []
    for c in range(NCORES):
        perm = np.arange(KPC) * NCORES + c  # global row ids, k-order
        perms.append(perm)
        qt_c = np.ascontiguousarray(Q[0][perm].T)
        prev_c = np.ascontiguousarray(
            prev[0][perm].reshape(2, 128, H, S).transpose(0, 2, 1, 3)
        )
        chan = perm // (P // D_IN)  # pred channel per row
        gcol_c = np.ascontiguousarray(
            gate[chan].reshape(2, 128, D_IN)
        )  # [m, r, B]
        stair_c = np.zeros((2, 128, 1024), np.float32)
        for m in range(2):
            so_ = 1024 * m if causal else 0
            allowed = mask2d[perm[128 * m : 128 * m + 128], so_ : so_ + 1024]
            stair_c[m][~allowed] = NEG
        in_maps.append(
            {
                "qt": qt_c,
                "kt": kt,
                "vt": vt,
                "wqt": wqt,
                "wkt": wkt,
                "wvt": wvt,
                "wot": wot,
                "prev": prev_c,
                "gatecol": gcol_c,
                "stairb": stair_c,
            }
        )

    nc = _get_program(causal)
    res = run_bass_kernel_spmd(nc, in_maps, core_ids=list(range(NCORES)))

    out = np.zeros((B, P, D), np.float32)
    attn = np.zeros((B, P, H, S), np.float32)
    raw = np.zeros((B, P, H, S), np.float32)
    for c in range(NCORES):
        r = res.results[c]
        perm = perms[c]
        out[0][perm] = r["out"].reshape(KPC, D)
        attn[0][perm] = r["attn"].transpose(0, 2, 1, 3).reshape(KPC, H, S)
        raw[0][perm] = r["raw"].transpose(0, 2, 1, 3).reshape(KPC, H, S)
    return out, attn, raw
